# revision 1
# baseline (speedup 1.0000x reference)
"""Expert-choice MoE kernel for 8 Trainium2 NeuronCores (Bass/Tile).

Distribution: expert-parallel, one expert per core.
  - gate: each core computes fp32 scores z = x_shard @ Wg for its 1/8 token
    shard, AllGather -> full (E, N) scores on every core.
  - top-k (k=2048 of N=8192) per expert: exact threshold via 32-step
    bisection on monotone uint32 keys of the fp32 scores, then index
    compaction with gpsimd sparse_gather.
  - dispatch: transposed dma_gather of the 2048 selected token rows (bf16).
  - expert FFN in bf16 (fp32 accumulation), erf-Gelu on the scalar engine,
    fp32 gate multiply on the way out.
  - combine: scatter rows into a per-core dense (N, H) buffer, ReduceScatter
    (add) across the 8 cores -> each core owns one token shard of the output.
"""

import sys

for _p in ("/opt/trn_rl_repo",):
    if _p not in sys.path:
        sys.path.insert(0, _p)

import numpy as np
import ml_dtypes

import concourse.bass as bass
import concourse.mybir as mybir
import concourse.tile as tile

# ---------------------------------------------------------------------------
# Patch: this walrus build rejects >1 sync-wait on the SP Drain that
# TileContext emits at kernel exit. Split the global-clock waits across
# several drains (1 wait each).
# ---------------------------------------------------------------------------
from concourse.vector_clock import ScopedClock

_MAX_DRAIN_WAITS = 1


def _patched_drain_and_barrier(self, tick_clock, wait_clock):
    nc = self.nc
    probe = nc.sync.drain()
    wait_clock.add_sem_waits(probe.ins, ScopedClock({None: tick_clock.global_clock}))
    si = probe.ins.sync_info
    waits = list(si.on_wait or []) if si is not None else []
    if len(waits) > _MAX_DRAIN_WAITS:
        probe.ins.sync_info = mybir.SyncInfo(
            on_wait=waits[:_MAX_DRAIN_WAITS],
            on_update=list(si.on_update or []),
        )
        for i in range(_MAX_DRAIN_WAITS, len(waits), _MAX_DRAIN_WAITS):
            extra = nc.sync.drain()
            extra.ins.sync_info = mybir.SyncInfo(
                on_wait=waits[i : i + _MAX_DRAIN_WAITS], on_update=[]
            )
    nc.all_engine_barrier()
    assert self.sems is not None
    popped = nc._tile_sem_poison_stack.pop()
    assert popped is self._sem_poison
    nc.clear_and_free_semaphores(list(self.sems.allocated().values()))
    nc.all_engine_barrier()


tile.TileContext._drain_and_barrier = _patched_drain_and_barrier

_WSPLIT_LIMIT = 1
_wsplit_ctr = [0]


def _split_excess_waits(nc, limit=_WSPLIT_LIMIT):
    """This walrus build encodes at most `limit` sync-wait commands per
    instruction; hoist excess waits onto same-engine Drain instructions
    inserted immediately before (per-engine streams execute in order)."""
    f = nc.m.functions[0]
    for b in f.blocks:
        insts = b.instructions
        out = []
        changed = False
        for inst in insts:
            si = getattr(inst, "sync_info", None)
            waits = list(si.on_wait or []) if si is not None else []
            eng = getattr(inst, "engine", None)
            if len(waits) > limit and eng is not None and \
                    eng != mybir.EngineType.Unassigned:
                keep = waits[-limit:]
                extra = waits[:-limit]
                for i in range(0, len(extra), limit):
                    d = mybir.InstDrain(
                        name=f"WSPLIT-{_wsplit_ctr[0]}", ins=[], outs=[])
                    _wsplit_ctr[0] += 1
                    d.engine = eng
                    d.sync_info = mybir.SyncInfo(
                        on_wait=extra[i:i + limit], on_update=[])
                    out.append(d)
                    nc.register_instruction(d, overwrite=True)
                inst.sync_info = mybir.SyncInfo(
                    on_wait=keep, on_update=list(si.on_update or []))
                changed = True
            out.append(inst)
        if changed:
            b.instructions = out

dt = mybir.dt
Alu = mybir.AluOpType
Act = mybir.ActivationFunctionType

N_CORES = 8

# problem dims (full size; can be shrunk for simulation)
FULL = dict(N=8192, H=1024, FF=4096, E=8, K=2048)


def build_moe_nc(N=8192, H=1024, FF=4096, E=8, K=2048, TOKG=512, act=None):
    """Build the SPMD Bass program (same program on all 8 cores)."""
    assert E == N_CORES
    P = N // N_CORES          # tokens per shard
    HC = H // 128             # h chunks
    FC = FF // 128            # ff chunks
    NG = K // TOKG            # token groups
    SUBS = TOKG // 128        # 128-token subtiles per group
    NCOLS = K // 128          # total 128-token subtiles
    ZF = N // 128             # free size of the [128, ZF] score layout
    assert K % TOKG == 0 and TOKG % 128 == 0 and P % 128 == 0
    if act is None:
        act = Act.Gelu
    NSTEP = min(512, H)

    nc = bass.Bass(num_devices=N_CORES)

    # ---- I/O ----
    xT_s = nc.dram_tensor("xT_s", [H, P], dt.float32, kind="ExternalInput")
    x_bf = nc.dram_tensor("x_bf", [N, H], dt.bfloat16, kind="ExternalInput")
    Wg_d = nc.dram_tensor("Wg", [H, E], dt.float32, kind="ExternalInput")
    W1_d = nc.dram_tensor("W1", [H, FF], dt.bfloat16, kind="ExternalInput")
    W2_d = nc.dram_tensor("W2", [FF, H], dt.bfloat16, kind="ExternalInput")
    b1_d = nc.dram_tensor("b1", [1, FF], dt.float32, kind="ExternalInput")
    b2_d = nc.dram_tensor("b2", [1, H], dt.float32, kind="ExternalInput")
    y_d = nc.dram_tensor("y", [P, H], dt.float32, kind="ExternalOutput")

    # ---- internal DRAM ----
    z_loc_d = nc.dram_tensor("z_loc", [E, P], dt.float32)
    z_e_d = nc.dram_tensor("z_e", [N_CORES, P], dt.float32)
    meta_d = nc.dram_tensor("meta", [K + 1, 2], dt.float32)
    dense_d = nc.dram_tensor("dense", [N, H], dt.float32)
    rs_out_d = nc.dram_tensor("rs_out", [P, H], dt.float32)

    groups = [list(range(N_CORES))]

    with tile.TileContext(nc) as tc:
        with (
            tc.tile_pool(name="const", bufs=1) as const_pool,
            tc.tile_pool(name="w", bufs=1) as w_pool,
            tc.tile_pool(name="psum1", bufs=2, space="PSUM") as psum1_pool,
            tc.tile_pool(name="psum2", bufs=2, space="PSUM") as psum2_pool,
            tc.tile_pool(name="ptrans", bufs=2, space="PSUM") as ptrans_pool,
        ):
            # ---------------- persistent constants ----------------
            ones1 = const_pool.tile([1, 128], dt.float32)
            nc.vector.memset(ones1[:], 1.0)

            # f - p iota: identity (f==p) and strict-lower-tri (f>p) masks
            fmp = const_pool.tile([128, 128], dt.int32)
            nc.gpsimd.iota(fmp[:], pattern=[[1, 128]], base=0,
                           channel_multiplier=-1)
            fmp_f = const_pool.tile([128, 128], dt.float32)
            nc.vector.tensor_copy(fmp_f[:], fmp[:])
            ident_bf = const_pool.tile([128, 128], dt.bfloat16)
            nc.vector.tensor_scalar(ident_bf[:], fmp_f[:], 0.0, None,
                                    op0=Alu.is_equal)
            ltri = const_pool.tile([128, 128], dt.float32)
            nc.vector.tensor_scalar(ltri[:], fmp_f[:], 0.0, None,
                                    op0=Alu.is_gt)
            ones128 = const_pool.tile([128, 128], dt.float32)
            nc.vector.memset(ones128[:], 1.0)

            # b2 broadcast [128, H] (constant along tokens)
            b2_sb = const_pool.tile([1, H], dt.float32)
            nc.sync.dma_start(b2_sb[:], b2_d[:])
            b2_ps = psum2_pool.tile([128, H], dt.float32, tag="ps2")
            for hh in range(0, H, NSTEP):
                nc.tensor.matmul(b2_ps[:, hh:hh + NSTEP], ones1[:],
                                 b2_sb[:, hh:hh + NSTEP], start=True, stop=True)
            b2_bcast = const_pool.tile([128, H], dt.float32)
            nc.vector.tensor_copy(b2_bcast[:], b2_ps[:])

            # b1 per-partition [128, FC]
            b1_pp = const_pool.tile([128, FC], dt.float32)
            nc.sync.dma_start(
                b1_pp[:], b1_d[:].rearrange("o (c p) -> (o p) c", p=128))

            zero_row = const_pool.tile([128, H], dt.float32)
            nc.vector.memset(zero_row[:], 0.0)

            # persistent routing outputs (filled by the gate phase)
            ids_pp = const_pool.tile([128, NCOLS], dt.int32)
            g_pp = const_pool.tile([128, NCOLS], dt.float32)

            # ---------------- weights (DMA overlaps the gate phase) ------
            w1_sb = w_pool.tile([128, HC, FF], dt.bfloat16)
            for ci in range(HC):
                nc.sync.dma_start(
                    w1_sb[:, ci, :], W1_d[ci * 128:(ci + 1) * 128, :])
            w2_sb = w_pool.tile([128, FC, H], dt.bfloat16)
            for fc in range(FC):
                nc.sync.dma_start(
                    w2_sb[:, fc, :], W2_d[fc * 128:(fc + 1) * 128, :])

            # ---------------- dense-buffer zero fill ----------------
            for i in range(N // 128):
                nc.sync.dma_start(dense_d[i * 128:(i + 1) * 128, :],
                                  zero_row[:])

            # ================= gate phase (scoped pool) ================
            with (
                tc.tile_pool(name="gate", bufs=1) as gate_pool,
                tc.tile_pool(name="small", bufs=2) as small_pool,
            ):
                # token-id iota in [128, ZF] layout (token = p*ZF + f)
                ids_i32 = gate_pool.tile([128, ZF], dt.int32)
                nc.gpsimd.iota(ids_i32[:], pattern=[[1, ZF]], base=0,
                               channel_multiplier=ZF)
                ids_f32 = gate_pool.tile([128, ZF], dt.float32)
                nc.vector.tensor_copy(ids_f32[:], ids_i32[:])

                xT_sb = gate_pool.tile([128, HC, P], dt.float32)
                nc.sync.dma_start(
                    xT_sb[:], xT_s[:].rearrange("(c p) t -> p c t", p=128))
                wg_sb = gate_pool.tile([128, HC, E], dt.float32)
                nc.sync.dma_start(
                    wg_sb[:], Wg_d[:].rearrange("(c p) e -> p c e", p=128))

                z_sb_loc = gate_pool.tile([E, P], dt.float32)
                for t0 in range(0, P, 512):
                    zw = min(512, P - t0)
                    z_ps = psum1_pool.tile([E, 512], dt.float32, tag="ps1")
                    for ci in range(HC):
                        nc.tensor.matmul(z_ps[:, :zw], wg_sb[:, ci, :],
                                         xT_sb[:, ci, t0:t0 + zw],
                                         start=(ci == 0), stop=(ci == HC - 1))
                    nc.vector.tensor_copy(z_sb_loc[:, t0:t0 + zw],
                                          z_ps[:, :zw])
                nc.sync.dma_start(z_loc_d[:], z_sb_loc[:])

                # core c receives every shard's scores for expert c
                nc.gpsimd.collective_compute(
                    "AllToAll", Alu.bypass, replica_groups=groups,
                    ins=[z_loc_d[:]], outs=[z_e_d[:]],
                )

                z_sb = gate_pool.tile([128, ZF], dt.float32)
                nc.sync.dma_start(
                    z_sb[:], z_e_d[:].rearrange("q t -> (q t)").rearrange(
                        "(p f) -> p f", p=128))

                # ---- fp32 value-space bisection for the k-th largest ----
                lo = gate_pool.tile([128, 1], dt.float32)
                hi = gate_pool.tile([128, 1], dt.float32)
                nc.vector.memset(lo[:], -1000.0)
                nc.vector.memset(hi[:], 1000.0)
                kf = float(K)
                for _ in range(45):
                    mid = small_pool.tile([128, 1], dt.float32, tag="mid")
                    nc.vector.tensor_tensor(mid[:], lo[:], hi[:], op=Alu.add)
                    nc.vector.tensor_scalar(mid[:], mid[:], 0.5, None,
                                            op0=Alu.mult)
                    cmpf = small_pool.tile([128, ZF], dt.float32, tag="cmpf")
                    nc.vector.tensor_scalar(cmpf[:], z_sb[:], mid[:, :1],
                                            None, op0=Alu.is_ge)
                    part = small_pool.tile([128, 1], dt.float32, tag="part")
                    nc.vector.tensor_reduce(part[:], cmpf[:],
                                            axis=mybir.AxisListType.X,
                                            op=Alu.add)
                    cnt_ps = psum1_pool.tile([128, 1], dt.float32, tag="ps1")
                    nc.tensor.matmul(cnt_ps[:], ones128[:], part[:],
                                     start=True, stop=True)
                    cnt = small_pool.tile([128, 1], dt.float32, tag="cnt")
                    nc.vector.tensor_copy(cnt[:], cnt_ps[:])
                    gemask = small_pool.tile([128, 1], dt.uint8, tag="gemask")
                    ltmask = small_pool.tile([128, 1], dt.uint8, tag="ltmask")
                    nc.vector.tensor_scalar(gemask[:], cnt[:], kf, None,
                                            op0=Alu.is_ge)
                    nc.vector.tensor_scalar(ltmask[:], cnt[:], kf, None,
                                            op0=Alu.is_lt)
                    nc.vector.copy_predicated(lo[:], gemask[:], mid[:])
                    nc.vector.copy_predicated(hi[:], ltmask[:], mid[:])

                # ---- selection mask + compact positions ----
                selmask = gate_pool.tile([128, ZF], dt.uint8)
                nc.vector.tensor_scalar(selmask[:], z_sb[:], lo[:, :1], None,
                                        op0=Alu.is_ge)
                maskf = gate_pool.tile([128, ZF], dt.float32)
                nc.vector.tensor_scalar(maskf[:], z_sb[:], lo[:, :1], None,
                                        op0=Alu.is_ge)

                # inclusive prefix along the free axis (log-shift)
                pa = gate_pool.tile([128, ZF], dt.float32)
                pb = gate_pool.tile([128, ZF], dt.float32)
                nc.vector.tensor_copy(pa[:], maskf[:])
                cur, nxt = pa, pb
                s = 1
                while s < ZF:
                    nc.vector.tensor_copy(nxt[:, :s], cur[:, :s])
                    nc.vector.tensor_tensor(nxt[:, s:], cur[:, s:],
                                            cur[:, :ZF - s], op=Alu.add)
                    cur, nxt = nxt, cur
                    s *= 2

                # row totals and exclusive cross-partition offsets
                offs_ps = psum1_pool.tile([128, 1], dt.float32, tag="ps1")
                nc.tensor.matmul(offs_ps[:], ltri[:], cur[:, ZF - 1:ZF],
                                 start=True, stop=True)
                pos = gate_pool.tile([128, ZF], dt.float32)
                # pos = rowpref + offs - 1
                offs_sb = gate_pool.tile([128, 1], dt.float32)
                nc.vector.tensor_copy(offs_sb[:], offs_ps[:])
                nc.vector.tensor_scalar(pos[:], cur[:], offs_sb[:, :1], None,
                                        op0=Alu.add)
                nc.vector.tensor_scalar(pos[:], pos[:], -1.0, None,
                                        op0=Alu.add)
                pos_i = gate_pool.tile([128, ZF], dt.int32)
                nc.vector.tensor_copy(pos_i[:], pos[:])
                posd = gate_pool.tile([128, ZF], dt.int32)
                nc.vector.memset(posd[:], K)
                nc.vector.copy_predicated(posd[:], selmask[:], pos_i[:])

                # meta rows (id, z) scattered to compact positions
                meta_sb = gate_pool.tile([128, ZF, 2], dt.float32)
                nc.vector.tensor_copy(meta_sb[:, :, 0], ids_f32[:])
                nc.vector.tensor_copy(meta_sb[:, :, 1], z_sb[:])
                for mi in range(ZF):
                    nc.gpsimd.indirect_dma_start(
                        out=meta_d[:],
                        out_offset=bass.IndirectOffsetOnAxis(
                            ap=posd[:, mi:mi + 1], axis=0),
                        in_=meta_sb[:, mi, :],
                        in_offset=None,
                    )

                # readback in per-partition layout: (p, s) = list[s*128 + p]
                meta_v = meta_d[0:K, :].rearrange("(s p) o -> p s o", p=128)
                idf = gate_pool.tile([128, NCOLS], dt.float32)
                nc.sync.dma_start(idf[:], meta_v[:, :, 0])
                nc.vector.tensor_copy(ids_pp[:], idf[:])
                zc = gate_pool.tile([128, NCOLS], dt.float32)
                nc.sync.dma_start(zc[:], meta_v[:, :, 1])
                nc.scalar.activation(g_pp[:], zc[:], Act.Sigmoid)

            # ================= FFN phase ================
            with (
                tc.tile_pool(name="ext", bufs=1) as ext_pool,
                tc.tile_pool(name="ex", bufs=1) as ex_pool,
                tc.tile_pool(name="hid", bufs=1) as hid_pool,
                tc.tile_pool(name="out", bufs=2) as out_pool,
            ):
                for g in range(NG):
                    # gather selected token rows (token-major)
                    ex_tok = ext_pool.tile([128, SUBS, H], dt.bfloat16,
                                           tag="ext")
                    for s in range(SUBS):
                        nc.gpsimd.indirect_dma_start(
                            out=ex_tok[:, s, :],
                            out_offset=None,
                            in_=x_bf[:],
                            in_offset=bass.IndirectOffsetOnAxis(
                                ap=ids_pp[:, g * SUBS + s:g * SUBS + s + 1],
                                axis=0),
                        )

                    # transpose to [h, tok] layout for the PE
                    ex_T = ex_pool.tile([128, HC, TOKG], dt.bfloat16,
                                        tag="ex")
                    for s in range(SUBS):
                        for ci in range(HC):
                            pt = ptrans_pool.tile([128, 128], dt.bfloat16,
                                                  tag="pt")
                            nc.tensor.transpose(
                                pt[:], ex_tok[:, s, ci * 128:(ci + 1) * 128],
                                ident_bf[:])
                            nc.vector.tensor_copy(
                                ex_T[:, ci, s * 128:(s + 1) * 128], pt[:])

                    hid_sb = hid_pool.tile([128, FC, TOKG], dt.bfloat16,
                                           tag="hid")
                    for fc in range(FC):
                        ps1 = psum1_pool.tile([128, TOKG], dt.float32,
                                              tag="ps1")
                        for ci in range(HC):
                            nc.tensor.matmul(
                                ps1[:], w1_sb[:, ci, fc * 128:(fc + 1) * 128],
                                ex_T[:, ci, :],
                                start=(ci == 0), stop=(ci == HC - 1))
                        nc.scalar.activation(hid_sb[:, fc, :], ps1[:], act,
                                             bias=b1_pp[:, fc:fc + 1])

                    for s in range(SUBS):
                        col = g * SUBS + s
                        pso = psum2_pool.tile([128, H], dt.float32, tag="ps2")
                        for hh in range(0, H, NSTEP):
                            for fc in range(FC):
                                nc.tensor.matmul(
                                    pso[:, hh:hh + NSTEP],
                                    hid_sb[:, fc, s * 128:(s + 1) * 128],
                                    w2_sb[:, fc, hh:hh + NSTEP],
                                    start=(fc == 0), stop=(fc == FC - 1))
                        out_sb = out_pool.tile([128, H], dt.float32, tag="osb")
                        nc.vector.tensor_tensor(out_sb[:], pso[:],
                                                b2_bcast[:], op=Alu.add)
                        nc.vector.tensor_scalar(out_sb[:], out_sb[:],
                                                g_pp[:, col:col + 1], None,
                                                op0=Alu.mult)
                        nc.gpsimd.indirect_dma_start(
                            out=dense_d[:],
                            out_offset=bass.IndirectOffsetOnAxis(
                                ap=ids_pp[:, col:col + 1], axis=0),
                            in_=out_sb[:],
                            in_offset=None,
                        )

                # ---------------- combine ----------------
                nc.gpsimd.collective_compute(
                    "ReduceScatter", Alu.add, replica_groups=groups,
                    ins=[dense_d[:]], outs=[rs_out_d[:]],
                )

                for i in range(P // 128):
                    ob = out_pool.tile([128, H], dt.float32, tag="osb")
                    nc.sync.dma_start(ob[:],
                                      rs_out_d[i * 128:(i + 1) * 128, :])
                    nc.sync.dma_start(y_d[i * 128:(i + 1) * 128, :], ob[:])

    _split_excess_waits(nc)
    return nc


# ---------------------------------------------------------------------------
# host-side sharding + execution
# ---------------------------------------------------------------------------

def make_in_maps(x, Wg, W1, b1, W2, b2, N=8192, H=1024):
    xt = np.ascontiguousarray(x.reshape(N, H).astype(np.float32))
    x_bf = xt.astype(ml_dtypes.bfloat16)
    P = N // N_CORES
    in_maps = []
    for c in range(N_CORES):
        shard = xt[c * P:(c + 1) * P, :]
        in_maps.append({
            "xT_s": np.ascontiguousarray(shard.T),
            "x_bf": x_bf,
            "Wg": np.ascontiguousarray(Wg.astype(np.float32)),
            "W1": np.ascontiguousarray(W1[c].astype(ml_dtypes.bfloat16)),
            "W2": np.ascontiguousarray(W2[c].astype(ml_dtypes.bfloat16)),
            "b1": np.ascontiguousarray(b1[c].reshape(1, -1).astype(np.float32)),
            "b2": np.ascontiguousarray(b2[c].reshape(1, -1).astype(np.float32)),
        })
    return in_maps


_NC_CACHE = {}


def kernel(x, Wg, W1, b1, W2, b2):
    x = np.asarray(x)
    B, L, H = x.shape
    N = B * L
    FF = W1.shape[2]
    key = (N, H, FF)
    if key not in _NC_CACHE:
        _NC_CACHE[key] = build_moe_nc(N=N, H=H, FF=FF)
    nc = _NC_CACHE[key]
    in_maps = make_in_maps(np.asarray(x), np.asarray(Wg), np.asarray(W1),
                           np.asarray(b1), np.asarray(W2), np.asarray(b2),
                           N=N, H=H)
    from concourse.bass_utils import run_bass_kernel_spmd
    res = run_bass_kernel_spmd(nc, in_maps, core_ids=list(range(N_CORES)),
                               trace=False)
    out = np.concatenate([res.results[c]["y"] for c in range(N_CORES)], axis=0)
    return out.reshape(B, L, H).astype(np.float32)

